# revision 14
# baseline (speedup 1.0000x reference)
"""Trainium2 Bass kernel for CustomMultiHeadAttention.

Problem: B=2, S=2048, E=1024, H=16 heads x 64 dim, fp32 in/out.
Returns (output [B,S,E], attn_weights [B,H,S,S]) like the torch module.

Sharding: 8 cores = 2 batches x 4 head-groups (4 heads each).  Each core
computes its group's Q/K/V projections (bf16 matmuls, fp32 accumulate),
softmax (exp on ACT in fp32, normalize on DVE), the context matmul, and a
partial out-projection over its 256 embed dims.  Host sums the 4 partials
per batch and adds bo.

Scores are computed twice on the PE - once [q,k] for the attn output and
once [k,q] to feed the context matmul - cheaper than any 16.8M-element
on-chip transpose.  Score matmuls are zero-padded to K=128 (a K=64
matmul runs at the cold-clock rate; zeros in the other head's rows cost
nothing and keep the clock warm).  The B (context) stream lags the A
(attn-output) stream by a few steps so ACT never starves across PSUM
pool transitions.
"""

import numpy as np
import ml_dtypes

EMBED = 1024
HEADS = 16
HD = 64
B = 2
S = 2048
SCALE = HD ** -0.5
NCORES = 8
GROUPS = 4          # head-groups per batch
HPG = HEADS // GROUPS  # heads per group = 4
GD = HPG * HD       # embed dims per group = 256

BF16 = ml_dtypes.bfloat16

TRACE = False        # set True (e.g. from test.py) to collect an NTFF profile
TMPDIR = None        # optional dir for NEFF/profile artifacts when tracing
LAST_RESULTS = None  # BassKernelResults of the last run

LAG = 4              # B-stream lag (steps) behind the A-stream

_COMPILED = None


def _build():
    import concourse.bass as bass
    import concourse.mybir as mybir
    import concourse.tile as tile
    from concourse import bacc
    from concourse.masks import make_identity

    f32 = mybir.dt.float32
    bf16 = mybir.dt.bfloat16
    Exp = mybir.ActivationFunctionType.Exp

    nc = bacc.Bacc(
        "TRN2",
        target_bir_lowering=False,
        debug=False,
        enable_asserts=False,
        num_devices=NCORES,
    )

    # ---- DRAM I/O (per core) ----
    xq_t = nc.dram_tensor("xq_t", [EMBED, S], bf16, kind="ExternalInput")
    xk_t = nc.dram_tensor("xk_t", [EMBED, S], bf16, kind="ExternalInput")
    xv_t = nc.dram_tensor("xv_t", [EMBED, S], bf16, kind="ExternalInput")
    wq_t = nc.dram_tensor("wq_t", [EMBED, GD], bf16, kind="ExternalInput")
    wk_t = nc.dram_tensor("wk_t", [EMBED, GD], bf16, kind="ExternalInput")
    wv_t = nc.dram_tensor("wv_t", [EMBED, GD], bf16, kind="ExternalInput")
    wo_t = nc.dram_tensor("wo_t", [GD, EMBED], bf16, kind="ExternalInput")
    bq_v = nc.dram_tensor("bq_v", [GD], f32, kind="ExternalInput")
    bk_v = nc.dram_tensor("bk_v", [GD], f32, kind="ExternalInput")
    bv_v = nc.dram_tensor("bv_v", [GD], f32, kind="ExternalInput")
    attn_o = nc.dram_tensor("attn_o", [HPG, S, S], f32, kind="ExternalOutput")
    out_o = nc.dram_tensor("out_o", [S, EMBED], f32, kind="ExternalOutput")

    KC = EMBED // 128  # 8 contraction chunks

    with tile.TileContext(nc) as tc:
        with (
            tc.tile_pool(name="const", bufs=1) as const,
            tc.tile_pool(name="wpool", bufs=1) as wpool,
            tc.tile_pool(name="xpool", bufs=4) as xpool,
            tc.tile_pool(name="qkv", bufs=1) as qkv,
            tc.tile_pool(name="work", bufs=2) as work,
            tc.tile_pool(name="ps", bufs=2, space="PSUM") as ps,
        ):
            # ---- constants ----
            ident = const.tile([128, 128], f32, name="ident")
            make_identity(nc, ident)
            ones1 = const.tile([1, 128], bf16, name="ones1")
            nc.gpsimd.memset(ones1, 1.0)
            bq_sb = const.tile([128, 2], f32, name="bq_sb")
            nc.sync.dma_start(bq_sb, bq_v.ap().rearrange("(m p) -> p m", p=128))
            bk_sb = const.tile([128, 2], f32, name="bk_sb")
            nc.sync.dma_start(bk_sb, bk_v.ap().rearrange("(m p) -> p m", p=128))
            bv_f = const.tile([1, GD], f32, name="bv_f")
            nc.sync.dma_start(bv_f, bv_v.ap().rearrange("(a n) -> a n", a=1))
            bv_sb = const.tile([1, GD], bf16, name="bv_sb")
            nc.vector.tensor_copy(bv_sb, bv_f)

            # ---- weights ----
            wq_sb = wpool.tile([128, KC, GD], bf16, name="wq_sb")
            nc.sync.dma_start(wq_sb, wq_t.ap().rearrange("(c p) m -> p c m", p=128))
            wk_sb = wpool.tile([128, KC, GD], bf16, name="wk_sb")
            nc.sync.dma_start(wk_sb, wk_t.ap().rearrange("(c p) m -> p c m", p=128))
            wv_sb = wpool.tile([128, KC, GD], bf16, name="wv_sb")
            nc.sync.dma_start(wv_sb, wv_t.ap().rearrange("(c p) m -> p c m", p=128))
            wo_sb = wpool.tile([128, 2, EMBED], bf16, name="wo_sb")
            nc.sync.dma_start(wo_sb, wo_t.ap().rearrange("(c p) n -> p c n", p=128))

            # x^T half-chunks [128, 1024] streamed just-in-time through a
            # small rotating pool (x is re-read from HBM per consumer pass;
            # DMA has headroom, SBUF does not).
            def stream_half(src, kc, half):
                t = xpool.tile([128, 1024], bf16, name="xc", tag="xc")
                nc.sync.dma_start(
                    t, src.ap()[kc * 128:(kc + 1) * 128,
                                half * 1024:(half + 1) * 1024])
                return t

            # ---- projection outputs ----
            qT_sb = qkv.tile([128, 2, S], bf16, name="qT_sb")
            kT_sb = qkv.tile([128, 2, S], bf16, name="kT_sb")
            qTz = [qkv.tile([128, 2, S], bf16, name=f"qTz{z}") for z in range(2)]
            kTz = [qkv.tile([128, 2, S], bf16, name=f"kTz{z}") for z in range(2)]
            v_sb = qkv.tile([128, S // 128, GD], bf16, name="v_sb")
            for z in range(2):
                zlo, zhi = (64, 128) if z == 0 else (0, 64)
                nc.gpsimd.memset(qTz[z][zlo:zhi, :, :], 0.0)
                nc.gpsimd.memset(kTz[z][zlo:zhi, :, :], 0.0)

            def project_qk_m(src, w_sb, b_sb, dst, dstz, m, pool, tag):
                # kc-outer: one x half-chunk resident at a time; the two
                # [128,1024] PSUM halves of this m-tile accumulate in
                # parallel.
                pt = {half: pool.tile([128, 1024], f32, name=f"pj{half}",
                                      tag=tag) for half in range(2)}
                for kc in range(KC):
                    for half in range(2):
                        xc = stream_half(src, kc, half)
                        for nn in range(2):
                            nc.tensor.matmul(
                                pt[half][:, nn * 512:(nn + 1) * 512],
                                w_sb[:, kc, m * 128:(m + 1) * 128],
                                xc[:, nn * 512:(nn + 1) * 512],
                                start=(kc == 0),
                                stop=(kc == KC - 1),
                            )
                for half in range(2):
                    sl = slice(half * 1024, (half + 1) * 1024)
                    nc.vector.tensor_scalar_add(
                        dst[:, m, sl], pt[half], b_sb[:, m:m + 1])
                    nc.vector.tensor_copy(
                        dstz[0][0:64, m, sl], dst[0:64, m, sl])
                    nc.vector.tensor_copy(
                        dstz[1][64:128, m, sl], dst[64:128, m, sl])

            # ---- attention step bodies ----
            recips = {}

            def a_step(p, qt):
                pair = (2 * p, 2 * p + 1)
                exp_t = {h: work.tile([128, S], f32, name="exp_t",
                                      tag="exp", bufs=3) for h in pair}
                sums = {h: [] for h in pair}
                for half in range(2):
                    sp = {}
                    for h in pair:
                        sp[h] = ps.tile([128, 1024], f32, name="sA",
                                        tag="stream")
                    for nn in range(2):
                        for h in pair:
                            m = h // 2
                            o = half * 1024 + nn * 512
                            nc.tensor.matmul(
                                sp[h][:, nn * 512:(nn + 1) * 512],
                                qTz[h % 2][:, m, qt * 128:(qt + 1) * 128],
                                kT_sb[:, m, o:o + 512],
                                start=True, stop=True,
                            )
                    for h in pair:
                        sacc = work.tile([128, 1], f32, name="sacc",
                                         tag="sums", bufs=8)
                        nc.scalar.activation(
                            exp_t[h][:, half * 1024:(half + 1) * 1024],
                            sp[h], Exp, accum_out=sacc)
                        sums[h].append(sacc)
                for h in pair:
                    st = work.tile([128, 1], f32, name="st", tag="sums",
                                   bufs=8)
                    nc.vector.tensor_add(st, sums[h][0], sums[h][1])
                    nc.vector.reciprocal(recips[h][:, qt:qt + 1], st)
                    attn_t = work.tile([128, S], f32, name="attn_t",
                                       tag="attn", bufs=4)
                    nc.vector.tensor_scalar_mul(attn_t, exp_t[h],
                                                recips[h][:, qt:qt + 1])
                    nc.sync.dma_start(
                        attn_o.ap()[h, qt * 128:(qt + 1) * 128, :], attn_t)
                # incremental transpose of the recip column into recipP
                if qt % 4 == 3:
                    for h in pair:
                        chunk = ps.tile([1, 512], f32, name="rch",
                                        tag="stream")
                        for i in range(4):
                            q4 = qt - 3 + i
                            nc.tensor.transpose(
                                chunk[0:1, i * 128:(i + 1) * 128],
                                recips[h][:, q4:q4 + 1], ident)
                        nc.vector.tensor_copy(
                            recipPs[h][0:1, (qt - 3) * 128:(qt + 1) * 128],
                            chunk)

            def b_step(p, kt, ctx_ps):
                pair = (2 * p, 2 * p + 1)
                expT = {h: work.tile([128, S], bf16, name="expT_t",
                                     tag="expT", bufs=3) for h in pair}
                for half in range(2):
                    sp = {}
                    for h in pair:
                        sp[h] = ps.tile([128, 1024], f32, name="sB",
                                        tag="stream")
                    for nn in range(2):
                        for h in pair:
                            m = h // 2
                            o = half * 1024 + nn * 512
                            nc.tensor.matmul(
                                sp[h][:, nn * 512:(nn + 1) * 512],
                                kTz[h % 2][:, m, kt * 128:(kt + 1) * 128],
                                qT_sb[:, m, o:o + 512],
                                start=True, stop=True,
                            )
                    for h in pair:
                        nc.scalar.activation(
                            expT[h][:, half * 1024:(half + 1) * 1024],
                            sp[h], Exp)
                for qc in range(4):
                    for h in pair:
                        hl = h - 2 * p
                        nc.tensor.matmul(
                            ctx_ps[hl * 64:hl * 64 + 64,
                                   qc * 512:(qc + 1) * 512],
                            v_sb[:, kt, h * 64:(h + 1) * 64],
                            expT[h][:, qc * 512:(qc + 1) * 512],
                            start=(kt == 0),
                            stop=(kt == 15),
                            tile_position=(0, hl * 64),
                            skip_group_check=True,
                        )

            # ---- schedule ----
            project_qk_m(xk_t, wk_sb, bk_sb, kT_sb, kTz, 0, ps, "stream")
            project_qk_m(xq_t, wq_sb, bq_sb, qT_sb, qTz, 0, ps, "stream")

            ctxT = []
            recipPs = {}
            for h in (0, 1):
                recips[h] = work.tile([128, 16], f32, name=f"recip{h}",
                                      tag="recip", bufs=4)
                recipPs[h] = work.tile([1, S], bf16, name=f"recipP{h}",
                                       tag="recipP", bufs=4)
            a_step(0, 0)
            a_step(0, 1)

            # v projection: 4 token-tiles per pass on scoped PSUM banks,
            # x^T(v) streamed (re-read per pass group)
            with tc.tile_pool(name="psv", bufs=1, space="PSUM") as psv:
                for ttg in range(4):
                    pv = psv.tile([128, S], f32, name="pv", tag="pv")
                    for tt4 in range(4):
                        nc.tensor.matmul(pv[:, tt4 * 512:tt4 * 512 + GD],
                                         ones1, bv_sb, start=True, stop=False)
                    for kc in range(KC):
                        xc = stream_half(xv_t, kc, ttg // 2)
                        for tt4 in range(4):
                            tt = ttg * 4 + tt4
                            loc = (tt % 8) * 128
                            nc.tensor.matmul(
                                pv[:, tt4 * 512:tt4 * 512 + GD],
                                xc[:, loc:loc + 128],
                                wv_sb[:, kc, :],
                                start=False,
                                stop=(kc == KC - 1),
                            )
                    for tt4 in range(4):
                        nc.vector.tensor_copy(
                            v_sb[:, ttg * 4 + tt4, :],
                            pv[:, tt4 * 512:tt4 * 512 + GD])

            a_step(0, 2)
            a_step(0, 3)

            # m=1 projections on scoped banks, interleaved with pair-0
            # A-steps that keep running on the base slots
            with tc.tile_pool(name="psm1", bufs=2, space="PSUM") as psm1:
                project_qk_m(xk_t, wk_sb, bk_sb, kT_sb, kTz, 1, psm1, "pm1")
                a_step(0, 4)
                project_qk_m(xq_t, wq_sb, bq_sb, qT_sb, qTz, 1, psm1, "pm1")
                a_step(0, 5)

            for p in range(2):
                pair = (2 * p, 2 * p + 1)
                pre = 6 if p == 0 else LAG
                if p == 1:
                    for h in pair:
                        recips[h] = work.tile([128, 16], f32,
                                              name=f"recip{h}",
                                              tag="recip", bufs=4)
                        recipPs[h] = work.tile([1, S], bf16,
                                               name=f"recipP{h}",
                                               tag="recipP", bufs=4)
                    for qt in range(pre):
                        a_step(p, qt)
                with tc.tile_pool(name=f"psP{p}", bufs=1,
                                  space="PSUM") as psP:
                    ctx_ps = psP.tile([128, S], f32, name="ctx_ps", tag="ctx")
                    for step in range(pre, 16 + pre):
                        if step < 16:
                            a_step(p, step)
                        b_step(p, step - pre, ctx_ps)

                    # --- normalize context for this pair ---
                    ctx_sb = qkv.tile([128, S], bf16, name=f"ctxT{p}",
                                      tag=f"ctxT{p}")
                    for h in pair:
                        hl = h - 2 * p
                        rB = work.tile([128, S], bf16, name="rB", tag="rB",
                                       bufs=1)
                        nc.gpsimd.partition_broadcast(rB, recipPs[h])
                        nc.vector.tensor_mul(
                            ctx_sb[hl * 64:hl * 64 + 64, :],
                            ctx_ps[hl * 64:hl * 64 + 64, :],
                            rB[hl * 64:hl * 64 + 64, :],
                        )
                    ctxT.append(ctx_sb)

            # ---- out projection ----
            for tt in range(S // 128):
                op = ps.tile([128, 1024], f32, name="op", tag="stream")
                for p in range(2):
                    for nn in range(2):
                        nc.tensor.matmul(
                            op[:, nn * 512:(nn + 1) * 512],
                            ctxT[p][:, tt * 128:(tt + 1) * 128],
                            wo_sb[:, p, nn * 512:(nn + 1) * 512],
                            start=(p == 0),
                            stop=(p == 1),
                        )
                out_sb = work.tile([128, 1024], f32, name="out_sb",
                                   tag="out", bufs=3)
                nc.vector.tensor_copy(out_sb, op)
                nc.sync.dma_start(out_o.ap()[tt * 128:(tt + 1) * 128, :], out_sb)

    nc.compile()
    return nc


def _get_compiled():
    global _COMPILED
    if _COMPILED is None:
        _COMPILED = _build()
    return _COMPILED


def kernel(query, key, value, attn_mask, Wq, bq, Wk, bk, Wv, bv, Wo, bo):
    global LAST_RESULTS
    q = np.asarray(query, np.float32)
    k = np.asarray(key, np.float32)
    v = np.asarray(value, np.float32)
    Wq = np.asarray(Wq, np.float32)
    Wk = np.asarray(Wk, np.float32)
    Wv = np.asarray(Wv, np.float32)
    Wo = np.asarray(Wo, np.float32)
    bq = np.asarray(bq, np.float32)
    bk = np.asarray(bk, np.float32)
    bv = np.asarray(bv, np.float32)
    bo = np.asarray(bo, np.float32)

    xT = {}
    for b in range(B):
        xT[("q", b)] = np.ascontiguousarray(q[b].T).astype(BF16)
        xT[("k", b)] = np.ascontiguousarray(k[b].T).astype(BF16)
        xT[("v", b)] = np.ascontiguousarray(v[b].T).astype(BF16)

    in_maps = []
    for c in range(NCORES):
        b = c // GROUPS
        g = c % GROUPS
        ds = slice(g * GD, (g + 1) * GD)
        in_maps.append({
            "xq_t": xT[("q", b)],
            "xk_t": xT[("k", b)],
            "xv_t": xT[("v", b)],
            "wq_t": np.ascontiguousarray((Wq[ds] * SCALE).T).astype(BF16),
            "wk_t": np.ascontiguousarray(Wk[ds].T).astype(BF16),
            "wv_t": np.ascontiguousarray(Wv[ds].T).astype(BF16),
            "wo_t": np.ascontiguousarray(Wo[:, ds].T).astype(BF16),
            "bq_v": np.ascontiguousarray(bq[ds] * SCALE),
            "bk_v": np.ascontiguousarray(bk[ds]),
            "bv_v": np.ascontiguousarray(bv[ds]),
        })

    from concourse.bass_utils import run_bass_kernel_spmd

    nc = _get_compiled()
    res = run_bass_kernel_spmd(nc, in_maps, core_ids=list(range(NCORES)),
                               trace=TRACE, tmpdir=TMPDIR)
    LAST_RESULTS = res
    results = res.results

    out = np.zeros((B, S, EMBED), np.float32)
    attn = np.empty((B, HEADS, S, S), np.float32)
    for c in range(NCORES):
        b = c // GROUPS
        g = c % GROUPS
        out[b] += np.asarray(results[c]["out_o"], np.float32)
        attn[b, g * HPG:(g + 1) * HPG] = np.asarray(results[c]["attn_o"],
                                                    np.float32)
    out += bo[None, None, :]
    return out, attn


# revision 19
# speedup vs baseline: 1.0456x; 1.0456x over previous
"""Trainium2 Bass kernel for CustomMultiHeadAttention.

Problem: B=2, S=2048, E=1024, H=16 heads x 64 dim, fp32 in/out.
Returns (output [B,S,E], attn_weights [B,H,S,S]) like the torch module.

Sharding: 8 cores = 2 batches x 4 head-groups (4 heads each).  Each core
computes its group's Q/K/V projections (bf16 matmuls, fp32 accumulate),
softmax (exp on ACT in fp32, normalize on DVE), the context matmul, and a
partial out-projection over its 256 embed dims.  Host sums the 4 partials
per batch and adds bo.

Scores are computed twice on the PE - once [q,k] for the attn output and
once [k,q] to feed the context matmul - cheaper than any 16.8M-element
on-chip transpose.  Score matmuls are zero-padded to K=128 (a K=64
matmul runs at the cold-clock rate; zeros in the other head's rows cost
nothing and keep the clock warm).  The B (context) stream lags the A
(attn-output) stream by a few steps so ACT never starves across PSUM
pool transitions.
"""

import numpy as np
import ml_dtypes

EMBED = 1024
HEADS = 16
HD = 64
B = 2
S = 2048
SCALE = HD ** -0.5
NCORES = 8
GROUPS = 4          # head-groups per batch
HPG = HEADS // GROUPS  # heads per group = 4
GD = HPG * HD       # embed dims per group = 256

BF16 = ml_dtypes.bfloat16

TRACE = False        # set True (e.g. from test.py) to collect an NTFF profile
TMPDIR = None        # optional dir for NEFF/profile artifacts when tracing
LAST_RESULTS = None  # BassKernelResults of the last run

LAG = 6              # B-stream lag (steps) behind the A-stream

_COMPILED = None


def _build():
    import concourse.bass as bass
    import concourse.mybir as mybir
    import concourse.tile as tile
    from concourse import bacc
    from concourse.masks import make_identity

    f32 = mybir.dt.float32
    bf16 = mybir.dt.bfloat16
    Exp = mybir.ActivationFunctionType.Exp

    nc = bacc.Bacc(
        "TRN2",
        target_bir_lowering=False,
        debug=False,
        enable_asserts=False,
        num_devices=NCORES,
    )

    # ---- DRAM I/O (per core) ----
    xq_t = nc.dram_tensor("xq_t", [EMBED, S], bf16, kind="ExternalInput")
    xk_t = nc.dram_tensor("xk_t", [EMBED, S], bf16, kind="ExternalInput")
    xv_t = nc.dram_tensor("xv_t", [EMBED, S], bf16, kind="ExternalInput")
    wq_t = nc.dram_tensor("wq_t", [EMBED, GD], bf16, kind="ExternalInput")
    wk_t = nc.dram_tensor("wk_t", [EMBED, GD], bf16, kind="ExternalInput")
    wv_t = nc.dram_tensor("wv_t", [EMBED, GD], bf16, kind="ExternalInput")
    wo_t = nc.dram_tensor("wo_t", [GD, EMBED], bf16, kind="ExternalInput")
    bq_v = nc.dram_tensor("bq_v", [GD], f32, kind="ExternalInput")
    bk_v = nc.dram_tensor("bk_v", [GD], f32, kind="ExternalInput")
    bv_v = nc.dram_tensor("bv_v", [GD], f32, kind="ExternalInput")
    attn_o = nc.dram_tensor("attn_o", [HPG, S, S], f32, kind="ExternalOutput")
    out_o = nc.dram_tensor("out_o", [S, EMBED], f32, kind="ExternalOutput")

    KC = EMBED // 128  # 8 contraction chunks

    with tile.TileContext(nc) as tc:
        with (
            tc.tile_pool(name="const", bufs=1) as const,
            tc.tile_pool(name="wpool", bufs=1) as wpool,
            tc.tile_pool(name="xpool", bufs=6) as xpool,
            tc.tile_pool(name="qkv", bufs=1) as qkv,
            tc.tile_pool(name="work", bufs=2) as work,
            tc.tile_pool(name="ps", bufs=2, space="PSUM") as ps,
        ):
            # ---- constants ----
            ident = const.tile([128, 128], f32, name="ident")
            make_identity(nc, ident)
            ones1 = const.tile([1, 128], bf16, name="ones1")
            nc.gpsimd.memset(ones1, 1.0)
            bq_sb = const.tile([128, 2], f32, name="bq_sb")
            nc.sync.dma_start(bq_sb, bq_v.ap().rearrange("(m p) -> p m", p=128))
            bk_sb = const.tile([128, 2], f32, name="bk_sb")
            nc.sync.dma_start(bk_sb, bk_v.ap().rearrange("(m p) -> p m", p=128))
            bv_f = const.tile([1, GD], f32, name="bv_f")
            nc.sync.dma_start(bv_f, bv_v.ap().rearrange("(a n) -> a n", a=1))
            bv_sb = const.tile([1, GD], bf16, name="bv_sb")
            nc.vector.tensor_copy(bv_sb, bv_f)

            # ---- weights ----
            wq_sb = wpool.tile([128, KC, GD], bf16, name="wq_sb")
            nc.sync.dma_start(wq_sb, wq_t.ap().rearrange("(c p) m -> p c m", p=128))
            wk_sb = wpool.tile([128, KC, GD], bf16, name="wk_sb")
            nc.sync.dma_start(wk_sb, wk_t.ap().rearrange("(c p) m -> p c m", p=128))
            wv_sb = wpool.tile([128, KC, GD], bf16, name="wv_sb")
            nc.sync.dma_start(wv_sb, wv_t.ap().rearrange("(c p) m -> p c m", p=128))
            wo_sb = wpool.tile([128, 2, EMBED], bf16, name="wo_sb")
            nc.sync.dma_start(wo_sb, wo_t.ap().rearrange("(c p) n -> p c n", p=128))

            # x^T half-chunks [128, 1024] streamed just-in-time through a
            # small rotating pool (x is re-read from HBM per consumer pass;
            # DMA has headroom, SBUF does not).
            def stream_half(src, kc, half):
                t = xpool.tile([128, 1024], bf16, name="xc", tag="xc")
                nc.sync.dma_start(
                    t, src.ap()[kc * 128:(kc + 1) * 128,
                                half * 1024:(half + 1) * 1024])
                return t

            # ---- projection outputs ----
            qT_sb = qkv.tile([128, 2, S], bf16, name="qT_sb")
            kT_sb = qkv.tile([128, 2, S], bf16, name="kT_sb")
            qTz = [qkv.tile([128, 2, S], bf16, name=f"qTz{z}") for z in range(2)]
            kTz = [qkv.tile([128, 2, S], bf16, name=f"kTz{z}") for z in range(2)]
            v_sb = qkv.tile([128, S // 128, GD], bf16, name="v_sb")
            for z in range(2):
                zlo, zhi = (64, 128) if z == 0 else (0, 64)
                nc.gpsimd.memset(qTz[z][zlo:zhi, :, :], 0.0)
                nc.gpsimd.memset(kTz[z][zlo:zhi, :, :], 0.0)

            def project_qk_m(src, w_sb, b_sb, dst, dstz, m, pool, tag,
                             mid=None):
                # kc-outer: one x half-chunk resident at a time; the two
                # [128,1024] PSUM halves of this m-tile accumulate in
                # parallel.
                pt = {half: pool.tile([128, 1024], f32, name=f"pj{half}",
                                      tag=tag) for half in range(2)}
                for kc in range(KC):
                    if kc == KC // 2 and mid is not None:
                        mid()
                    for half in range(2):
                        xc = stream_half(src, kc, half)
                        for nn in range(2):
                            nc.tensor.matmul(
                                pt[half][:, nn * 512:(nn + 1) * 512],
                                w_sb[:, kc, m * 128:(m + 1) * 128],
                                xc[:, nn * 512:(nn + 1) * 512],
                                start=(kc == 0),
                                stop=(kc == KC - 1),
                            )
                for half in range(2):
                    sl = slice(half * 1024, (half + 1) * 1024)
                    nc.vector.tensor_scalar_add(
                        dst[:, m, sl], pt[half], b_sb[:, m:m + 1])
                    nc.vector.tensor_copy(
                        dstz[0][0:64, m, sl], dst[0:64, m, sl])
                    nc.vector.tensor_copy(
                        dstz[1][64:128, m, sl], dst[64:128, m, sl])

            # ---- attention step bodies ----
            recips = {}

            def a_step(p, qt):
                pair = (2 * p, 2 * p + 1)
                exp_t = {h: work.tile([128, S], f32, name="exp_t",
                                      tag="exp", bufs=3) for h in pair}
                sums = {h: [] for h in pair}
                for half in range(2):
                    sp = {}
                    for h in pair:
                        sp[h] = ps.tile([128, 1024], f32, name="sA",
                                        tag="stream")
                    for nn in range(2):
                        for h in pair:
                            m = h // 2
                            o = half * 1024 + nn * 512
                            nc.tensor.matmul(
                                sp[h][:, nn * 512:(nn + 1) * 512],
                                qTz[h % 2][:, m, qt * 128:(qt + 1) * 128],
                                kT_sb[:, m, o:o + 512],
                                start=True, stop=True,
                            )
                    for h in pair:
                        sacc = work.tile([128, 1], f32, name="sacc",
                                         tag="sums", bufs=8)
                        nc.scalar.activation(
                            exp_t[h][:, half * 1024:(half + 1) * 1024],
                            sp[h], Exp, accum_out=sacc)
                        sums[h].append(sacc)
                for h in pair:
                    st = work.tile([128, 1], f32, name="st", tag="sums",
                                   bufs=8)
                    nc.vector.tensor_add(st, sums[h][0], sums[h][1])
                    nc.vector.reciprocal(recips[h][:, qt:qt + 1], st)
                    attn_t = work.tile([128, S], f32, name="attn_t",
                                       tag="attn", bufs=4)
                    nc.vector.tensor_scalar_mul(attn_t, exp_t[h],
                                                recips[h][:, qt:qt + 1])
                    nc.sync.dma_start(
                        attn_o.ap()[h, qt * 128:(qt + 1) * 128, :], attn_t)
            def recip_chunk(h, g):
                # transpose recip columns 4g..4g+3 into recipP [1, S]
                chunk = ps.tile([1, 512], f32, name="rch", tag="stream")
                for i in range(4):
                    q4 = g * 4 + i
                    nc.tensor.transpose(
                        chunk[0:1, i * 128:(i + 1) * 128],
                        recips[h][:, q4:q4 + 1], ident)
                nc.vector.tensor_copy(
                    recipPs[h][0:1, g * 512:(g + 1) * 512], chunk)

            def b_step(p, kt, ctx_ps):
                pair = (2 * p, 2 * p + 1)
                expT = {h: work.tile([128, S], bf16, name="expT_t",
                                     tag="expT", bufs=3) for h in pair}
                for half in range(2):
                    sp = {}
                    for h in pair:
                        sp[h] = ps.tile([128, 1024], f32, name="sB",
                                        tag="stream")
                    for nn in range(2):
                        for h in pair:
                            m = h // 2
                            o = half * 1024 + nn * 512
                            nc.tensor.matmul(
                                sp[h][:, nn * 512:(nn + 1) * 512],
                                kTz[h % 2][:, m, kt * 128:(kt + 1) * 128],
                                qT_sb[:, m, o:o + 512],
                                start=True, stop=True,
                            )
                    for h in pair:
                        nc.scalar.activation(
                            expT[h][:, half * 1024:(half + 1) * 1024],
                            sp[h], Exp)
                for qc in range(4):
                    for h in pair:
                        hl = h - 2 * p
                        nc.tensor.matmul(
                            ctx_ps[hl * 64:hl * 64 + 64,
                                   qc * 512:(qc + 1) * 512],
                            v_sb[:, kt, h * 64:(h + 1) * 64],
                            expT[h][:, qc * 512:(qc + 1) * 512],
                            start=(kt == 0),
                            stop=(kt == 15),
                            tile_position=(0, hl * 64),
                            skip_group_check=True,
                        )

            # ---- schedule ----
            project_qk_m(xk_t, wk_sb, bk_sb, kT_sb, kTz, 0, ps, "stream")
            project_qk_m(xq_t, wq_sb, bq_sb, qT_sb, qTz, 0, ps, "stream")

            ctxT = []
            recipPs = {}
            for h in (0, 1):
                recips[h] = work.tile([128, 16], f32, name=f"recip{h}",
                                      tag="recip", bufs=4)
                recipPs[h] = work.tile([1, S], bf16, name=f"recipP{h}",
                                       tag="recipP", bufs=4)
            a_step(0, 0)
            a_step(0, 1)

            # v projection: 4 token-tiles per pass on scoped PSUM banks,
            # x^T(v) streamed (re-read per pass group)
            with tc.tile_pool(name="psv", bufs=1, space="PSUM") as psv:
                for ttg in range(4):
                    pv = psv.tile([128, S], f32, name="pv", tag="pv")
                    for tt4 in range(4):
                        nc.tensor.matmul(pv[:, tt4 * 512:tt4 * 512 + GD],
                                         ones1, bv_sb, start=True, stop=False)
                    for kc in range(KC):
                        xc = stream_half(xv_t, kc, ttg // 2)
                        for tt4 in range(4):
                            tt = ttg * 4 + tt4
                            loc = (tt % 8) * 128
                            nc.tensor.matmul(
                                pv[:, tt4 * 512:tt4 * 512 + GD],
                                xc[:, loc:loc + 128],
                                wv_sb[:, kc, :],
                                start=False,
                                stop=(kc == KC - 1),
                            )
                    for tt4 in range(4):
                        nc.vector.tensor_copy(
                            v_sb[:, ttg * 4 + tt4, :],
                            pv[:, tt4 * 512:tt4 * 512 + GD])

            a_step(0, 2)
            a_step(0, 3)

            # m=1 projections on scoped banks, interleaved with pair-0
            # A-steps that keep running on the base slots
            with tc.tile_pool(name="psm1", bufs=2, space="PSUM") as psm1:
                project_qk_m(xk_t, wk_sb, bk_sb, kT_sb, kTz, 1, psm1, "pm1",
                             mid=lambda: a_step(0, 4))
                a_step(0, 5)
                project_qk_m(xq_t, wq_sb, bq_sb, qT_sb, qTz, 1, psm1, "pm1",
                             mid=lambda: a_step(0, 6))
                a_step(0, 7)
                a_step(0, 8)

            for p in range(2):
                pair = (2 * p, 2 * p + 1)
                pre = 9 if p == 0 else LAG
                if p == 1:
                    for h in pair:
                        recips[h] = work.tile([128, 16], f32,
                                              name=f"recip{h}",
                                              tag="recip", bufs=4)
                        recipPs[h] = work.tile([1, S], bf16,
                                               name=f"recipP{h}",
                                               tag="recipP", bufs=4)
                    for qt in range(pre):
                        a_step(p, qt)
                with tc.tile_pool(name=f"psP{p}", bufs=1,
                                  space="PSUM") as psP:
                    ctx_ps = psP.tile([128, S], f32, name="ctx_ps", tag="ctx")
                    # recip chunk groups spread across the late B steps
                    chunk_sched = {}
                    for j, bkt in enumerate([10, 11, 12, 12, 13, 14, 15, 15]):
                        chunk_sched.setdefault(bkt, []).append(
                            (pair[j // 4], j % 4))
                    for step in range(pre, 16 + pre):
                        if step < 16:
                            a_step(p, step)
                        bkt = step - pre
                        b_step(p, bkt, ctx_ps)
                        for args in chunk_sched.get(bkt, []):
                            recip_chunk(*args)

                    # --- normalize context for this pair ---
                    ctx_sb = qkv.tile([128, S], bf16, name=f"ctxT{p}",
                                      tag=f"ctxT{p}")
                    for h in pair:
                        hl = h - 2 * p
                        rB = work.tile([128, S], bf16, name="rB", tag="rB",
                                       bufs=1)
                        nc.gpsimd.partition_broadcast(rB, recipPs[h])
                        nc.vector.tensor_mul(
                            ctx_sb[hl * 64:hl * 64 + 64, :],
                            ctx_ps[hl * 64:hl * 64 + 64, :],
                            rB[hl * 64:hl * 64 + 64, :],
                        )
                    ctxT.append(ctx_sb)

            # ---- out projection ----
            for tt in range(S // 128):
                op = ps.tile([128, 1024], f32, name="op", tag="stream")
                for p in range(2):
                    for nn in range(2):
                        nc.tensor.matmul(
                            op[:, nn * 512:(nn + 1) * 512],
                            ctxT[p][:, tt * 128:(tt + 1) * 128],
                            wo_sb[:, p, nn * 512:(nn + 1) * 512],
                            start=(p == 0),
                            stop=(p == 1),
                        )
                out_sb = work.tile([128, 1024], f32, name="out_sb",
                                   tag="out", bufs=3)
                nc.vector.tensor_copy(out_sb, op)
                nc.sync.dma_start(out_o.ap()[tt * 128:(tt + 1) * 128, :], out_sb)

    nc.compile()
    return nc


def _get_compiled():
    global _COMPILED
    if _COMPILED is None:
        _COMPILED = _build()
    return _COMPILED


def kernel(query, key, value, attn_mask, Wq, bq, Wk, bk, Wv, bv, Wo, bo):
    global LAST_RESULTS
    q = np.asarray(query, np.float32)
    k = np.asarray(key, np.float32)
    v = np.asarray(value, np.float32)
    Wq = np.asarray(Wq, np.float32)
    Wk = np.asarray(Wk, np.float32)
    Wv = np.asarray(Wv, np.float32)
    Wo = np.asarray(Wo, np.float32)
    bq = np.asarray(bq, np.float32)
    bk = np.asarray(bk, np.float32)
    bv = np.asarray(bv, np.float32)
    bo = np.asarray(bo, np.float32)

    xT = {}
    for b in range(B):
        xT[("q", b)] = np.ascontiguousarray(q[b].T).astype(BF16)
        xT[("k", b)] = np.ascontiguousarray(k[b].T).astype(BF16)
        xT[("v", b)] = np.ascontiguousarray(v[b].T).astype(BF16)

    in_maps = []
    for c in range(NCORES):
        b = c // GROUPS
        g = c % GROUPS
        ds = slice(g * GD, (g + 1) * GD)
        in_maps.append({
            "xq_t": xT[("q", b)],
            "xk_t": xT[("k", b)],
            "xv_t": xT[("v", b)],
            "wq_t": np.ascontiguousarray((Wq[ds] * SCALE).T).astype(BF16),
            "wk_t": np.ascontiguousarray(Wk[ds].T).astype(BF16),
            "wv_t": np.ascontiguousarray(Wv[ds].T).astype(BF16),
            "wo_t": np.ascontiguousarray(Wo[:, ds].T).astype(BF16),
            "bq_v": np.ascontiguousarray(bq[ds] * SCALE),
            "bk_v": np.ascontiguousarray(bk[ds]),
            "bv_v": np.ascontiguousarray(bv[ds]),
        })

    from concourse.bass_utils import run_bass_kernel_spmd

    nc = _get_compiled()
    res = run_bass_kernel_spmd(nc, in_maps, core_ids=list(range(NCORES)),
                               trace=TRACE, tmpdir=TMPDIR)
    LAST_RESULTS = res
    results = res.results

    out = np.zeros((B, S, EMBED), np.float32)
    attn = np.empty((B, HEADS, S, S), np.float32)
    for c in range(NCORES):
        b = c // GROUPS
        g = c % GROUPS
        out[b] += np.asarray(results[c]["out_o"], np.float32)
        attn[b, g * HPG:(g + 1) * HPG] = np.asarray(results[c]["attn_o"],
                                                    np.float32)
    out += bo[None, None, :]
    return out, attn
